# revision 1
# baseline (speedup 1.0000x reference)
"""Transformer policy kernel builder for TRN2 (Bass/Tile), feature-major design.

Per core (data-parallel over batch): BC=8 batches x S=256 -> T=2048 tokens.
D=512 (4 chunks), H=8 heads (HD=64), FF=2048 (16 chunks), L layers.

Residual stream xT: 4 tiles [128, 2048] feature-major f32r.
Everything else processed per 512-token segment (= 2 batches):
  qkv(seg) -> attention(2 batches x 8 heads) -> Wo+resid+LN1 -> FFN+resid+LN2.
Head: Wp1->LN+gelu->Wp2->LN+gelu->Wp3->tanh->(scale,bias)->transpose->out.
"""
import math
import contextlib
import numpy as np

import concourse.bass as bass
import concourse.bacc as bacc
import concourse.tile as tile
from concourse import mybir

F32 = mybir.dt.float32
F32R = mybir.dt.float32r
AF = mybir.ActivationFunctionType
ALU = mybir.AluOpType

BC = 8
S = 256
T = BC * S
OBS = 96
ACT_DIM = 29
D = 512
H = 8
HD = 64
FF = 2048
NC_D = D // 128
NC_FF = FF // 128
TT = 512
NSEG = T // TT
EPS = 1e-5
L_MAX = 8


def _nz(a):
    return a is not None and bool(np.any(np.asarray(a) != 0))


def _ng(a):
    return a is not None and bool(np.any(np.asarray(a) != 1))


def build(inputs, n_layers=8, emit_head=True, dbg_x=False):
    """inputs: dict of full np arrays (reference naming). Returns (nc, extra_in_map)."""
    nc = bacc.Bacc("TRN2", target_bir_lowering=False, debug=False)

    flags = dict(
        bin_=_nz(inputs["b_in"]), gin=_ng(inputs["g_in"]), bein=_nz(inputs["be_in"]),
        bq=_nz(inputs["bq"]), bk=_nz(inputs["bk"]), bv=_nz(inputs["bv"]), bo=_nz(inputs["bo"]),
        g1=_ng(inputs["g1"]), be1=_nz(inputs["be1"]), b1=_nz(inputs["b1"]), b2=_nz(inputs["b2"]),
        g2=_ng(inputs["g2"]), be2=_nz(inputs["be2"]),
        bp1=_nz(inputs["bp1"]), gp1=_ng(inputs["gp1"]), bep1=_nz(inputs["bep1"]),
        bp2=_nz(inputs["bp2"]), gp2=_ng(inputs["gp2"]), bep2=_nz(inputs["bep2"]),
        bp3=_nz(inputs["bp3"]), asc=_ng(inputs["action_scale"]), abi=_nz(inputs["action_bias"]),
    )

    def din(name, shape, dt=F32R):
        return nc.dram_tensor(name, shape, dt, kind="ExternalInput").ap()

    obs_d = din("observations", (BC, S, OBS), F32)
    win_d = din("W_in", (OBS, D))
    wq_d = din("Wq", (L_MAX, D, D)); wk_d = din("Wk", (L_MAX, D, D))
    wv_d = din("Wv", (L_MAX, D, D)); wo_d = din("Wo", (L_MAX, D, D))
    w1_d = din("W1", (L_MAX, D, FF)); w2_d = din("W2", (L_MAX, FF, D))
    wp1_d = din("Wp1", (D, D // 2)); wp2_d = din("Wp2", (D // 2, D // 4))
    wp3_d = din("Wp3", (D // 4, ACT_DIM))
    ident_d = din("IDENT", (128, 128), F32)
    ones_d = din("ONES", (128, 128))
    pet_d = din("PET", (D, S))
    out_d = nc.dram_tensor("OUT", (T, ACT_DIM), F32, kind="ExternalOutput").ap()
    if dbg_x:
        xdbg_d = nc.dram_tensor("XDBG", (D, T), F32R, kind="ExternalOutput").ap()

    extra = {
        "IDENT": np.eye(128, dtype=np.float32),
        "ONES": np.ones((128, 128), np.float32),
    }
    pos = np.arange(S, dtype=np.float32)[:, None]
    div = np.exp(np.arange(0, D, 2, dtype=np.float32) * (-math.log(10000.0) / D))
    pe = np.zeros((S, D), dtype=np.float32)
    pe[:, 0::2] = np.sin(pos * div)
    pe[:, 1::2] = np.cos(pos * div)
    extra["PET"] = np.ascontiguousarray(pe.T)

    def vec_tensor(name, arr):
        a = np.asarray(arr, np.float32).reshape(-1)
        n = a.size // 128
        extra[name] = np.ascontiguousarray(a.reshape(n, 128).T)
        return din(name, (128, n), F32)

    dv = {}
    for key, nm in [("bq", "BQ"), ("bk", "BK"), ("bo", "BO"), ("b1", "B1"), ("b2", "B2"),
                    ("b_in", "BIN"), ("g_in", "GIN"), ("be_in", "BEIN"),
                    ("g1", "G1"), ("be1", "BE1"), ("g2", "G2"), ("be2", "BE2"),
                    ("bp1", "BP1"), ("gp1", "GP1"), ("bep1", "BEP1"),
                    ("bp2", "BP2"), ("gp2", "GP2"), ("bep2", "BEP2")]:
        fkey = {"b_in": "bin_", "g_in": "gin", "be_in": "bein"}.get(key, key)
        if flags[fkey]:
            dv[nm] = vec_tensor(nm + "v", inputs[key])
    if flags["bv"]:
        extra["BVr"] = np.asarray(inputs["bv"], np.float32).reshape(L_MAX, D)
        dv["BV"] = din("BVr", (L_MAX, D))

    def vec29(name, arr):
        a = np.zeros((128, 1), np.float32)
        a[:ACT_DIM, 0] = np.asarray(arr, np.float32).reshape(-1)
        extra[name] = a
        return din(name, (128, 1), F32)
    if flags["bp3"]:
        dv["BP3"] = vec29("BP3v", inputs["bp3"])
    if flags["asc"]:
        dv["ASC"] = vec29("ASCv", inputs["action_scale"])
    if flags["abi"]:
        dv["ABI"] = vec29("ABIv", inputs["action_bias"])

    scale = 1.0 / math.sqrt(HD)

    with tile.TileContext(nc) as tc:
        with contextlib.ExitStack() as ctx:
            P = {}
            P["persist"] = ctx.enter_context(tc.tile_pool(name="persist", bufs=1))
            P["wpool"] = ctx.enter_context(tc.tile_pool(name="wpool", bufs=1))
            P["xpool"] = ctx.enter_context(tc.tile_pool(name="xpool", bufs=1))
            P["segt"] = ctx.enter_context(tc.tile_pool(name="segt", bufs=1))
            P["exps"] = ctx.enter_context(tc.tile_pool(name="exps", bufs=2))
            P["sq"] = ctx.enter_context(tc.tile_pool(name="sq", bufs=1))
            P["scratch"] = ctx.enter_context(tc.tile_pool(name="scratch", bufs=2))
            P["mini"] = ctx.enter_context(tc.tile_pool(name="mini", bufs=1))
            P["bcast"] = ctx.enter_context(tc.tile_pool(name="bcast", bufs=1))
            P["rbs"] = ctx.enter_context(tc.tile_pool(name="rbs", bufs=2))
            P["rec"] = ctx.enter_context(tc.tile_pool(name="rec", bufs=1))
            P["hpool"] = ctx.enter_context(tc.tile_pool(name="hpool", bufs=1))
            P["pbig"] = ctx.enter_context(tc.tile_pool(name="pbig", bufs=4, space="PSUM"))
            P["psmall"] = ctx.enter_context(tc.tile_pool(name="psmall", bufs=4, space="PSUM"))

            # ---------------- constants ----------------
            ident = P["persist"].tile([128, 128], F32, tag="ident")
            nc.sync.dma_start(out=ident, in_=ident_d[:, :])
            ones_w = 128 if flags["bv"] else 8
            ones = P["persist"].tile([128, ones_w], F32R, tag="ones")
            nc.sync.dma_start(out=ones, in_=ones_d[:, 0:ones_w])
            ones_col = ones[:, 0:1]

            peT = P["persist"].tile([128, NC_D * S], F32R, tag="peT")
            for c in range(NC_D):
                nc.sync.dma_start(out=peT[:, c * S:(c + 1) * S],
                                  in_=pet_d[c * 128:(c + 1) * 128, :])

            vt = {}
            for nm, d in dv.items():
                if nm == "BV":
                    t = P["persist"].tile([1, L_MAX * D], F32R, tag="c_BV")
                    for l in range(L_MAX):
                        nc.sync.dma_start(out=t[:, l * D:(l + 1) * D], in_=d[l:l + 1, :])
                else:
                    t = P["persist"].tile([128, d.shape[1]], F32, tag=f"c_{nm}")
                    nc.sync.dma_start(out=t, in_=d[:, :])
                vt[nm] = t

            # ---------------- big tiles ----------------
            xT = [[P["xpool"].tile([128, TT], F32R, tag=f"xT{c}_{s}", name=f"xT{c}_{s}")
                   for s in range(NSEG)] for c in range(NC_D)]
            seg_q = [P["segt"].tile([128, TT], F32R, tag=f"sq{c}", name=f"sq{c}") for c in range(NC_D)]
            seg_k = [P["segt"].tile([128, TT], F32R, tag=f"sk{c}", name=f"sk{c}") for c in range(NC_D)]
            seg_v = [P["segt"].tile([128, TT], F32R, tag=f"sv{c}", name=f"sv{c}") for c in range(NC_D)]
            seg_o = [P["segt"].tile([128, TT], F32R, tag=f"so{c}", name=f"so{c}") for c in range(NC_D)]
            hT = [P["hpool"].tile([128, TT], F32R, tag=f"hT{m}", name=f"hT{m}")
                  for m in range(8)]

            # weights (persistent slots, re-DMAed per layer)
            wq_t = [P["wpool"].tile([128, D], F32R, tag=f"wq{k}", name=f"wq{k}") for k in range(NC_D)]
            wk_t = [P["wpool"].tile([128, D], F32R, tag=f"wk{k}", name=f"wk{k}") for k in range(NC_D)]
            wv_t = [P["wpool"].tile([128, D], F32R, tag=f"wv{k}", name=f"wv{k}") for k in range(NC_D)]
            wo_t = [P["wpool"].tile([128, D], F32R, tag=f"wo{k}", name=f"wo{k}") for k in range(NC_D)]
            w1_t = [P["wpool"].tile([128, FF], F32R, tag=f"w1{k}", name=f"w1{k}") for k in range(NC_D)]
            w2_t = [P["wpool"].tile([128, D], F32R, tag=f"w2{k}", name=f"w2{k}") for k in range(NC_FF)]
            # aliases into w1 slots for input/head weights
            win_v = w1_t[3][:, 256:768]              # [128(96 used), 512]
            wp1_v = [w1_t[k][:, 0:D // 2] for k in range(NC_D)]
            wp2_v = [w1_t[k][:, 768:768 + D // 4] for k in range(2)]
            wp3_v = w1_t[2][:, 768:768 + ACT_DIM]

            # ---------------- helpers ----------------
            def ap_vec(nm, idx):
                t = vt.get(nm)
                return t[:, idx:idx + 1] if t is not None else None

            def layernorm(chunk_aps, nfeat, g_fn, b_fn, gelu=False):
                nch = len(chunk_aps)
                sums = P["psmall"].tile([1, TT], F32, tag="sm")
                sumsq = P["psmall"].tile([1, TT], F32, tag="sm")
                for c in range(nch):
                    xc = chunk_aps[c]
                    sqt = P["sq"].tile([128, TT], F32R, tag="sqt")
                    nc.vector.tensor_mul(sqt, xc, xc)
                    nc.tensor.matmul(sums, ones_col, xc,
                                     start=(c == 0), stop=(c == nch - 1))
                    nc.tensor.matmul(sumsq, ones_col, sqt,
                                     start=(c == 0), stop=(c == nch - 1))
                m = P["mini"].tile([1, TT], F32, tag="m")
                nc.scalar.mul(m, sums, 1.0 / nfeat)
                msq = P["mini"].tile([1, TT], F32, tag="msq")
                nc.vector.tensor_mul(msq, m, m)
                e2 = P["mini"].tile([1, TT], F32, tag="e2")
                # e2 = (sumsq + n*eps) * (1/n) = sumsq/n + eps
                nc.vector.tensor_scalar(out=e2, in0=sumsq, scalar1=float(nfeat) * EPS,
                                        scalar2=1.0 / nfeat, op0=ALU.add, op1=ALU.mult)
                nc.vector.tensor_sub(e2, e2, msq)
                nc.vector.reciprocal_approx_fast(out=msq, in_=e2)
                nc.scalar.sqrt(e2, msq)         # e2 = rstd
                M = P["bcast"].tile([128, TT], F32, tag="Mb")
                nc.gpsimd.partition_broadcast(M, m)
                R = P["bcast"].tile([128, TT], F32, tag="Rb")
                nc.gpsimd.partition_broadcast(R, e2)
                for c in range(nch):
                    xc = chunk_aps[c]
                    g_ap, b_ap = g_fn(c), b_fn(c)
                    nc.vector.tensor_sub(xc, xc, M)
                    if gelu:
                        nc.vector.tensor_mul(xc, xc, R)
                        nc.scalar.activation(xc, xc, AF.Gelu,
                                             bias=b_ap if b_ap is not None else 0.0,
                                             scale=g_ap if g_ap is not None else 1.0)
                    else:
                        nc.vector.scalar_tensor_tensor(
                            xc, xc, g_ap if g_ap is not None else 1.0, R,
                            ALU.mult, ALU.mult)
                        if b_ap is not None:
                            nc.scalar.activation(xc, xc, AF.Identity, bias=b_ap, scale=1.0)

            def proj_fm(w_tiles, in_aps, out_aps, bias_fn, copy_dve=False,
                        act=None, resid=False, kpart=128, pool="pbig"):
                """feature-major projection: out[mc] = W.T @ in ( + bias ), psum-wise."""
                n_out = len(out_aps)
                n_in = len(in_aps)
                for mc in range(n_out):
                    ps = P[pool].tile([128, TT], F32, tag="pb")
                    for kc in range(n_in):
                        nc.tensor.matmul(
                            ps, w_tiles[kc][0:kpart, mc * 128:(mc + 1) * 128],
                            in_aps[kc][0:kpart, :],
                            start=(kc == 0), stop=(kc == n_in - 1))
                    b_ap = bias_fn(mc) if bias_fn is not None else None
                    if resid:
                        xc = out_aps[mc]
                        nc.vector.scalar_tensor_tensor(
                            xc, ps, b_ap if b_ap is not None else 0.0, xc,
                            ALU.add, ALU.add)
                    elif act == "gelu":
                        nc.scalar.activation(out_aps[mc], ps, AF.Gelu,
                                             bias=b_ap if b_ap is not None else 0.0,
                                             scale=1.0)
                    elif copy_dve and b_ap is None:
                        nc.vector.tensor_copy(out_aps[mc], ps)
                    else:
                        nc.scalar.activation(out_aps[mc], ps, AF.Identity,
                                             bias=b_ap if b_ap is not None else 0.0,
                                             scale=1.0)

            # ---------------- input stage ----------------
            obs_flat = obs_d.rearrange("b s f -> (b s) f")
            nc.sync.dma_start(out=win_v[0:OBS, :], in_=win_d[:, :])
            for seg in range(NSEG):
                obsT = seg_k[0]  # [96, 512] region used
                for ts in range(4):
                    t0 = seg * TT + ts * 128
                    ot = P["scratch"].tile([128, OBS], F32, tag="obs_in")
                    nc.sync.dma_start(out=ot, in_=obs_flat[t0:t0 + 128, :])
                    tp = P["pbig"].tile([OBS, 128], F32, tag="pb")
                    nc.tensor.transpose(tp, ot, ident)
                    nc.scalar.copy(obsT[0:OBS, ts * 128:(ts + 1) * 128], tp)
                xs = [xT[c][seg][:, :] for c in range(NC_D)]
                proj_fm([win_v], [obsT[:, :]], xs,
                        (lambda mc: ap_vec("BIN", mc)) if flags["bin_"] else None,
                        kpart=OBS)
                layernorm(xs, D,
                          (lambda c: ap_vec("GIN", c)) if flags["gin"] else (lambda c: None),
                          (lambda c: ap_vec("BEIN", c)) if flags["bein"] else (lambda c: None),
                          gelu=True)
                for c in range(NC_D):
                    xc = xs[c]
                    nc.vector.tensor_add(
                        xc.rearrange("p (b s) -> p b s", s=S),
                        xc.rearrange("p (b s) -> p b s", s=S),
                        peT[:, c * S:(c + 1) * S].unsqueeze(1).broadcast_to([128, TT // S, S]))

            # ---------------- layers ----------------
            def load_qkv_w(lx):
                for k in range(NC_D):
                    nc.sync.dma_start(out=wq_t[k], in_=wq_d[lx, k * 128:(k + 1) * 128, :])
                    nc.sync.dma_start(out=wk_t[k], in_=wk_d[lx, k * 128:(k + 1) * 128, :])
                    nc.sync.dma_start(out=wv_t[k], in_=wv_d[lx, k * 128:(k + 1) * 128, :])

            qkv_pre = False
            for l in range(n_layers):
                if not qkv_pre:
                    load_qkv_w(l)
                for k in range(NC_D):
                    nc.sync.dma_start(out=wo_t[k], in_=wo_d[l, k * 128:(k + 1) * 128, :])
                    nc.sync.dma_start(out=w1_t[k], in_=w1_d[l, k * 128:(k + 1) * 128, :])
                for k in range(NC_FF):
                    nc.sync.dma_start(out=w2_t[k], in_=w2_d[l, k * 128:(k + 1) * 128, :])

                def emit_qkv_attn(seg, lx=None):
                    lq = l if lx is None else lx
                    xs = [xT[c][seg][:, :] for c in range(NC_D)]
                    # q/k projections (feature-major)
                    proj_fm(wq_t, xs, [t[:, :] for t in seg_q],
                            (lambda mc: ap_vec("BQ", lq * 4 + mc)) if flags["bq"] else None,
                            copy_dve=True)
                    proj_fm(wk_t, xs, [t[:, :] for t in seg_k],
                            (lambda mc: ap_vec("BK", lq * 4 + mc)) if flags["bk"] else None,
                            copy_dve=True)
                    # v projection (token-major)
                    for ts in range(4):
                        vp = P["pbig"].tile([128, D], F32, tag="pb")
                        for kc in range(NC_D):
                            nc.tensor.matmul(
                                vp, xT[kc][seg][:, ts * 128:(ts + 1) * 128],
                                wv_t[kc],
                                start=(kc == 0), stop=(kc == NC_D - 1) and not flags["bv"])
                        if flags["bv"]:
                            nc.tensor.matmul(vp, ones[0:1, 0:128],
                                             vt["BV"][:, lq * D:(lq + 1) * D],
                                             start=False, stop=True)
                        nc.vector.tensor_copy(seg_v[ts][:, :], vp)
                    # attention: 2 batches x 4 head-pairs
                    for b2 in range(2):
                        bcol = b2 * S
                        for c in range(NC_D):  # head pair (2c, 2c+1) lives in chunk c
                            es2 = []
                            for hh in range(2):
                                roff = hh * HD
                                scp = P["pbig"].tile([128, 2 * S], F32, tag="pb")
                                for kc in range(2):
                                    nc.tensor.matmul(
                                        scp[:, kc * S:(kc + 1) * S],
                                        seg_k[c][roff:roff + HD,
                                                 bcol + kc * 128: bcol + (kc + 1) * 128],
                                        seg_q[c][roff:roff + HD, bcol:bcol + S],
                                        start=True, stop=True)
                                esh = P["exps"].tile([128, 2 * S], F32R, tag="es",
                                                     name=f"es{hh}")
                                nc.scalar.activation(esh, scp, AF.Exp, bias=0.0,
                                                     scale=scale)
                                es2.append(esh)
                            aux = P["psmall"].tile([1, 2 * S], F32, tag="sm")
                            otp = [P["psmall"].tile([HD, S], F32, tag="sm", name=f"otp{hh}")
                                   for hh in range(2)]
                            for hh in range(2):
                                h = 2 * c + hh
                                for kc in range(2):
                                    nc.tensor.matmul(
                                        aux[:, hh * S:(hh + 1) * S],
                                        ones_col,
                                        es2[hh][:, kc * S:(kc + 1) * S],
                                        start=(kc == 0), stop=(kc == 1))
                                for kc in range(2):
                                    nc.tensor.matmul(
                                        otp[hh],
                                        seg_v[b2 * 2 + kc][:, h * HD:(h + 1) * HD],
                                        es2[hh][:, kc * S:(kc + 1) * S],
                                        start=(kc == 0), stop=(kc == 1))
                            rec = P["rec"].tile([1, 2 * S], F32, tag="rec")
                            nc.vector.reciprocal_approx_fast(out=rec, in_=aux)
                            rbs = P["rbs"].tile([128, 2 * S], F32, tag="rbs")
                            nc.gpsimd.partition_broadcast(rbs, rec)
                            # even head: rows 0:64 directly; odd head: via SBUF->SBUF
                            # DMA (engines cannot shift partitions; DMA can)
                            nc.vector.tensor_mul(
                                seg_o[c][0:HD, bcol:bcol + S],
                                otp[0], rbs[0:HD, 0:S])
                            otmp = P["rbs"].tile([HD, S], F32R, tag="otmp")
                            nc.vector.tensor_mul(otmp, otp[1], rbs[0:HD, S:2 * S])
                            nc.sync.dma_start(out=seg_o[c][HD:128, bcol:bcol + S],
                                              in_=otmp)

                if not qkv_pre:
                    emit_qkv_attn(0)
                qkv_pre = False
                for seg in range(NSEG):
                    xs = [xT[c][seg][:, :] for c in range(NC_D)]
                    # Wo + residual + LN1
                    proj_fm(wo_t, [t[:, :] for t in seg_o], xs,
                            (lambda mc: ap_vec("BO", l * 4 + mc)) if flags["bo"] else None,
                            resid=True)
                    layernorm(xs, D,
                              (lambda c: ap_vec("G1", l * 4 + c)) if flags["g1"] else (lambda c: None),
                              (lambda c: ap_vec("BE1", l * 4 + c)) if flags["be1"] else (lambda c: None))
                    # software pipeline: next segment's qkv+attention fills the
                    # PE while this segment's LN1/FFN epilogues run
                    if seg + 1 < NSEG:
                        emit_qkv_attn(seg + 1)
                    elif l + 1 < n_layers:
                        load_qkv_w(l + 1)
                        emit_qkv_attn(0, lx=l + 1)
                        qkv_pre = True
                    # FFN
                    # two-wave FFN at full N=512: 8 hidden tiles reused per
                    # wave; W2 psums accumulate across both waves
                    w2ps = [P["pbig"].tile([128, TT], F32, tag="pb", name=f"w2ps{m}")
                            for m in range(NC_D)]
                    for wave in range(2):
                        for mc8 in range(8):
                            mc = wave * 8 + mc8
                            ps = P["psmall"].tile([128, TT], F32, tag="sm")
                            for kc in range(NC_D):
                                nc.tensor.matmul(
                                    ps, w1_t[kc][:, mc * 128:(mc + 1) * 128], xs[kc],
                                    start=(kc == 0), stop=(kc == NC_D - 1))
                            nc.scalar.activation(
                                hT[mc8][:, :], ps, AF.Gelu,
                                bias=ap_vec("B1", l * 16 + mc) if flags["b1"] else 0.0,
                                scale=1.0)
                        for mcD in range(NC_D):
                            for kc8 in range(8):
                                kc = wave * 8 + kc8
                                nc.tensor.matmul(
                                    w2ps[mcD], w2_t[kc][:, mcD * 128:(mcD + 1) * 128],
                                    hT[kc8][:, :],
                                    start=(kc == 0), stop=(kc == NC_FF - 1))
                    for mcD in range(NC_D):
                        nc.vector.scalar_tensor_tensor(
                            xs[mcD], w2ps[mcD],
                            ap_vec("B2", l * 4 + mcD) if flags["b2"] else 0.0,
                            xs[mcD], ALU.add, ALU.add)
                    layernorm(xs, D,
                              (lambda c: ap_vec("G2", l * 4 + c)) if flags["g2"] else (lambda c: None),
                              (lambda c: ap_vec("BE2", l * 4 + c)) if flags["be2"] else (lambda c: None))

            if dbg_x:
                for c in range(NC_D):
                    for s in range(NSEG):
                        nc.sync.dma_start(
                            out=xdbg_d[c * 128:(c + 1) * 128, s * TT:(s + 1) * TT],
                            in_=xT[c][s][:, :])

            # ---------------- head ----------------
            if emit_head:
                for k in range(NC_D):
                    nc.sync.dma_start(out=wp1_v[k], in_=wp1_d[k * 128:(k + 1) * 128, :])
                for k in range(2):
                    nc.sync.dma_start(out=wp2_v[k], in_=wp2_d[k * 128:(k + 1) * 128, :])
                nc.sync.dma_start(out=wp3_v, in_=wp3_d[:, :])
                for seg in range(NSEG):
                    xs = [xT[c][seg][:, :] for c in range(NC_D)]
                    y1 = [seg_q[0][:, :], seg_q[1][:, :]]
                    proj_fm(wp1_v, xs, y1,
                            (lambda mc: ap_vec("BP1", mc)) if flags["bp1"] else None)
                    layernorm(y1, D // 2,
                              (lambda c: ap_vec("GP1", c)) if flags["gp1"] else (lambda c: None),
                              (lambda c: ap_vec("BEP1", c)) if flags["bep1"] else (lambda c: None),
                              gelu=True)
                    y2 = [seg_q[2][:, :]]
                    proj_fm(wp2_v, y1, y2,
                            (lambda mc: ap_vec("BP2", 0)) if flags["bp2"] else None)
                    layernorm(y2, D // 4,
                              (lambda c: ap_vec("GP2", 0)) if flags["gp2"] else (lambda c: None),
                              (lambda c: ap_vec("BEP2", 0)) if flags["bep2"] else (lambda c: None),
                              gelu=True)
                    actp = P["psmall"].tile([ACT_DIM, TT], F32, tag="sm")
                    nc.tensor.matmul(actp, wp3_v, y2[0], start=True, stop=True)
                    actT = P["mini"].tile([ACT_DIM, TT], F32, tag="actT")
                    nc.scalar.activation(actT[0:ACT_DIM, :], actp, AF.Tanh,
                                         bias=vt["BP3"][0:ACT_DIM, 0:1] if flags["bp3"] else 0.0,
                                         scale=1.0)
                    if flags["asc"] or flags["abi"]:
                        nc.scalar.activation(
                            actT[0:ACT_DIM, :], actT[0:ACT_DIM, :], AF.Identity,
                            bias=vt["ABI"][0:ACT_DIM, 0:1] if flags["abi"] else 0.0,
                            scale=vt["ASC"][0:ACT_DIM, 0:1] if flags["asc"] else 1.0)
                    for ts in range(4):
                        tp = P["psmall"].tile([128, ACT_DIM], F32, tag="sm")
                        nc.tensor.transpose(tp, actT[0:ACT_DIM, ts * 128:(ts + 1) * 128],
                                            ident[0:ACT_DIM, 0:ACT_DIM])
                        ob = P["scratch"].tile([128, ACT_DIM], F32, tag="ob")
                        nc.vector.tensor_copy(ob, tp)
                        nc.sync.dma_start(
                            out=out_d[seg * TT + ts * 128: seg * TT + (ts + 1) * 128, :],
                            in_=ob)

    nc.compile()
    return nc, extra

# ======================================================================
# Self-contained kernel entry point: takes FULL inputs, shards batch over
# 8 NeuronCores (data-parallel), runs the Bass kernel, gathers output.
# ======================================================================
from concourse.bass_utils import run_bass_kernel_spmd

N_CORES = 8


def kernel(**inputs):
    inputs = {k: np.asarray(v) for k, v in inputs.items()}
    nc, extra = build(inputs, n_layers=8, emit_head=True, dbg_x=False)

    base = dict(extra)
    for k in ["W_in", "Wq", "Wk", "Wv", "Wo", "W1", "W2", "Wp1", "Wp2", "Wp3"]:
        base[k] = np.ascontiguousarray(inputs[k], dtype=np.float32)

    obs = np.asarray(inputs["observations"], np.float32)
    n_b = obs.shape[0]
    per = n_b // N_CORES
    in_maps = []
    for c in range(N_CORES):
        m = dict(base)
        m["observations"] = np.ascontiguousarray(obs[c * per:(c + 1) * per])
        in_maps.append(m)

    last_err = None
    for attempt in range(4):
        try:
            res = run_bass_kernel_spmd(nc, in_maps, core_ids=list(range(N_CORES)),
                                       trace=False)
            outs = [res.results[c]["OUT"].reshape(per, S, ACT_DIM)
                    for c in range(N_CORES)]
            return np.concatenate(outs, axis=0)
        except Exception as e:  # transient NRT_EXEC_UNIT_UNRECOVERABLE etc.
            last_err = e
            import time as _time
            _time.sleep(3.0 * (attempt + 1))
    raise last_err



# revision 29
# speedup vs baseline: 1.0792x; 1.0792x over previous
"""Transformer policy kernel builder for TRN2 (Bass/Tile), feature-major, bf16.

Per core (data-parallel over batch): BC=8 batches x S=256 -> T=2048 tokens.
D=512 (4 chunks), H=8 heads (HD=64), FF=2048 (16 chunks), L layers.

v2 design vs v1 baseline:
  - bf16 weights + activations (psum stays f32): halves DVE time, weight DMA,
    and SBUF pressure.
  - softmax denominator folded into the attnV stationary via a
    [V_even | ones | zeros(63) | V_odd] 192-col per-head-pair layout: the even
    head's matmul yields O_even in psum rows 0:64 and the denominator in row
    64; the odd head's yields the denominator in row 0 and O_odd in rows
    64:128 -- no separate row-sum matmuls and no SBUF->SBUF partition-shift
    DMA for the odd head.
  - scores matmuls for a head pair emitted back-to-back: they target PE row
    groups (0,0)/(64,0) and can overlap in the array.
  - double-buffered layer weights (bf16 makes them fit): layer l+1 weights DMA
    while layer l computes.
  - single-wave FFN (16 hidden tiles) with per-output-chunk sequential W2
    accumulation: psum pressure low, next-seg Wo projection emitted before
    LN2 so the PE never drains while the LN vector chain runs.
  - head emitted phase-major (all Wp1+LN, all Wp2+LN, all Wp3+tanh) with
    per-segment buffers so the 4 segments pipeline.
"""
import math
import contextlib
import numpy as np
import ml_dtypes

import concourse.bass as bass
import concourse.bacc as bacc
import concourse.tile as tile
from concourse import mybir

F32 = mybir.dt.float32
F32R = mybir.dt.float32r
BF16 = mybir.dt.bfloat16
AF = mybir.ActivationFunctionType
ALU = mybir.AluOpType

BC = 8
S = 256
T = BC * S
OBS = 96
ACT_DIM = 29
D = 512
H = 8
HD = 64
FF = 2048
NC_D = D // 128
NC_FF = FF // 128
TT = 512
NSEG = T // TT
EPS = 1e-5
L_MAX = 8
VXW = 192          # per head-pair stride in the attnV stationary layout
BF = ml_dtypes.bfloat16


def _nz(a):
    return a is not None and bool(np.any(np.asarray(a) != 0))


def _ng(a):
    return a is not None and bool(np.any(np.asarray(a) != 1))


def build(inputs, n_layers=8, emit_head=True, dbg_x=False, dbg_att=False):
    """inputs: dict of full np arrays (reference naming). Returns (nc, extra_in_map)."""
    nc = bacc.Bacc("TRN2", target_bir_lowering=False, debug=False)

    flags = dict(
        bin_=_nz(inputs["b_in"]), gin=_ng(inputs["g_in"]), bein=_nz(inputs["be_in"]),
        bq=_nz(inputs["bq"]), bk=_nz(inputs["bk"]), bv=_nz(inputs["bv"]), bo=_nz(inputs["bo"]),
        g1=_ng(inputs["g1"]), be1=_nz(inputs["be1"]), b1=_nz(inputs["b1"]), b2=_nz(inputs["b2"]),
        g2=_ng(inputs["g2"]), be2=_nz(inputs["be2"]),
        bp1=_nz(inputs["bp1"]), gp1=_ng(inputs["gp1"]), bep1=_nz(inputs["bep1"]),
        bp2=_nz(inputs["bp2"]), gp2=_ng(inputs["gp2"]), bep2=_nz(inputs["bep2"]),
        bp3=_nz(inputs["bp3"]), asc=_ng(inputs["action_scale"]), abi=_nz(inputs["action_bias"]),
    )

    def din(name, shape, dt=BF16):
        return nc.dram_tensor(name, shape, dt, kind="ExternalInput").ap()

    obs_d = din("observations", (BC, S, OBS))
    win_d = din("W_in", (OBS, D))
    wq_d = din("Wq", (L_MAX, D, D)); wk_d = din("Wk", (L_MAX, D, D))
    wv_d = din("Wv", (L_MAX, D, D)); wo_d = din("Wo", (L_MAX, D, D))
    w1_d = din("W1", (L_MAX, D, FF)); w2_d = din("W2", (L_MAX, FF, D))
    wp1_d = din("Wp1", (D, D // 2)); wp2_d = din("Wp2", (D // 2, D // 4))
    wp3_d = din("Wp3", (D // 4, ACT_DIM))
    ident_d = din("IDENT", (128, 128))
    ones_d = din("ONES", (128, 128))
    pet_d = din("PET", (D, S))
    out_d = nc.dram_tensor("OUT", (T, ACT_DIM), F32, kind="ExternalOutput").ap()
    if dbg_x:
        xdbg_d = nc.dram_tensor("XDBG", (D, T), BF16, kind="ExternalOutput").ap()
    if dbg_att:
        adbg_d = nc.dram_tensor("ADBG", (3, D, TT), BF16, kind="ExternalOutput").ap()
        vdbg_d = nc.dram_tensor("VDBG", (4, 128, NC_D * VXW), BF16, kind="ExternalOutput").ap()
        rdbg_d = nc.dram_tensor("RDBG", (2, 128, S), BF16, kind="ExternalOutput").ap()

    extra = {
        "IDENT": np.eye(128, dtype=BF),
        "ONES": np.ones((128, 128), BF),
    }
    # selector for broadcasting softmax reciprocals: out rows 0:64 get rec
    # row 64 (even head), rows 64:128 get rec row 0 (odd head)
    sel = np.zeros((128, 128), np.float32)
    sel[64, 0:64] = 1.0
    sel[0, 64:128] = 1.0
    extra["SELM"] = sel.astype(BF)
    selm_d = din("SELM", (128, 128), BF16)
    pos = np.arange(S, dtype=np.float32)[:, None]
    div = np.exp(np.arange(0, D, 2, dtype=np.float32) * (-math.log(10000.0) / D))
    pe = np.zeros((S, D), dtype=np.float32)
    pe[:, 0::2] = np.sin(pos * div)
    pe[:, 1::2] = np.cos(pos * div)
    extra["PET"] = np.ascontiguousarray(pe.T).astype(BF)

    def vec_tensor(name, arr):
        a = np.asarray(arr, np.float32).reshape(-1)
        n = a.size // 128
        extra[name] = np.ascontiguousarray(a.reshape(n, 128).T)
        return din(name, (128, n), F32)

    dv = {}
    for key, nm in [("bq", "BQ"), ("bk", "BK"), ("bo", "BO"), ("b1", "B1"), ("b2", "B2"),
                    ("b_in", "BIN"), ("g_in", "GIN"), ("be_in", "BEIN"),
                    ("g1", "G1"), ("be1", "BE1"), ("g2", "G2"), ("be2", "BE2"),
                    ("bp1", "BP1"), ("gp1", "GP1"), ("bep1", "BEP1"),
                    ("bp2", "BP2"), ("gp2", "GP2"), ("bep2", "BEP2")]:
        fkey = {"b_in": "bin_", "g_in": "gin", "be_in": "bein"}.get(key, key)
        if flags[fkey]:
            dv[nm] = vec_tensor(nm + "v", inputs[key])
    if flags["bv"]:
        extra["BVr"] = np.asarray(inputs["bv"], np.float32).astype(BF).reshape(L_MAX, D)
        dv["BV"] = din("BVr", (L_MAX, D))

    def vec29(name, arr):
        a = np.zeros((128, 1), np.float32)
        a[:ACT_DIM, 0] = np.asarray(arr, np.float32).reshape(-1)
        extra[name] = a
        return din(name, (128, 1), F32)
    if flags["bp3"]:
        dv["BP3"] = vec29("BP3v", inputs["bp3"])
    if flags["asc"]:
        dv["ASC"] = vec29("ASCv", inputs["action_scale"])
    if flags["abi"]:
        dv["ABI"] = vec29("ABIv", inputs["action_bias"])

    scale = 1.0 / math.sqrt(HD)

    with tile.TileContext(nc) as tc:
        with contextlib.ExitStack() as ctx:
            P = {}
            P["persist"] = ctx.enter_context(tc.tile_pool(name="persist", bufs=1))
            P["wpool"] = ctx.enter_context(tc.tile_pool(name="wpool", bufs=1))
            P["xpool"] = ctx.enter_context(tc.tile_pool(name="xpool", bufs=1))
            P["segt"] = ctx.enter_context(tc.tile_pool(name="segt", bufs=1))
            P["exps"] = ctx.enter_context(tc.tile_pool(name="exps", bufs=4))
            P["sq"] = ctx.enter_context(tc.tile_pool(name="sq", bufs=2))
            P["scratch"] = ctx.enter_context(tc.tile_pool(name="scratch", bufs=4))
            P["mini"] = ctx.enter_context(tc.tile_pool(name="mini", bufs=2))
            P["bcast"] = ctx.enter_context(tc.tile_pool(name="bcast", bufs=2))
            P["rbs"] = ctx.enter_context(tc.tile_pool(name="rbs", bufs=4))
            P["rec"] = ctx.enter_context(tc.tile_pool(name="rec", bufs=2))
            P["hpool"] = ctx.enter_context(tc.tile_pool(name="hpool", bufs=1))
            P["headt"] = ctx.enter_context(tc.tile_pool(name="headt", bufs=4))
            # PSUM is bank-granular (8 x 2KB/partition):
            # pb 5x[128,512]f32 (5) + st 2x[1,512]f32 (2) + tpb 1 (1) = 8 banks
            P["pbig"] = ctx.enter_context(tc.tile_pool(name="pbig", bufs=4, space="PSUM"))
            P["ptp"] = ctx.enter_context(tc.tile_pool(name="ptp", bufs=1, space="PSUM"))
            P["pstat"] = ctx.enter_context(tc.tile_pool(name="pstat", bufs=2, space="PSUM"))

            # ---------------- constants ----------------
            ident = P["persist"].tile([128, 128], BF16, tag="ident")
            nc.sync.dma_start(out=ident, in_=ident_d[:, :])
            ones_w = 128 if flags["bv"] else 8
            ones = P["persist"].tile([128, ones_w], BF16, tag="ones")
            nc.sync.dma_start(out=ones, in_=ones_d[:, 0:ones_w])
            ones_col = ones[:, 0:1]
            selm = P["persist"].tile([128, 128], BF16, tag="selm")
            nc.sync.dma_start(out=selm, in_=selm_d[:, :])
            # denominator staging tiles: only partitions 0 and 64 are ever
            # written; the rest must be zero so the selector matmul contracts
            # cleanly
            denf_t = [P["persist"].tile([128, S], BF16, tag=f"denf{i}", name=f"denf{i}")
                      for i in range(2)]
            for i in range(2):
                nc.vector.memset(denf_t[i], 0.0)

            peT = P["persist"].tile([128, NC_D * S], BF16, tag="peT")
            for c in range(NC_D):
                nc.sync.dma_start(out=peT[:, c * S:(c + 1) * S],
                                  in_=pet_d[c * 128:(c + 1) * 128, :])

            vt = {}
            for nm, d in dv.items():
                if nm == "BV":
                    t = P["persist"].tile([1, L_MAX * D], BF16, tag="c_BV")
                    for l in range(L_MAX):
                        nc.sync.dma_start(out=t[:, l * D:(l + 1) * D], in_=d[l:l + 1, :])
                else:
                    t = P["persist"].tile([128, d.shape[1]], F32, tag=f"c_{nm}")
                    nc.sync.dma_start(out=t, in_=d[:, :])
                vt[nm] = t

            # ---------------- big tiles ----------------
            xT = [[P["xpool"].tile([128, TT], BF16, tag=f"xT{c}_{s}", name=f"xT{c}_{s}")
                   for s in range(NSEG)] for c in range(NC_D)]
            seg_q = [P["segt"].tile([128, TT], BF16, tag=f"sq{c}", name=f"sq{c}") for c in range(NC_D)]
            seg_k = [P["segt"].tile([128, TT], BF16, tag=f"sk{c}", name=f"sk{c}") for c in range(NC_D)]
            seg_o = [P["segt"].tile([128, TT], BF16, tag=f"so{c}", name=f"so{c}") for c in range(NC_D)]
            # attnV stationary: 4 ts-tiles x [128 tok, 4 pairs x 192]
            seg_vx = [P["segt"].tile([128, NC_D * VXW], BF16, tag=f"svx{t_}", name=f"svx{t_}")
                      for t_ in range(4)]
            hT = [P["hpool"].tile([128, TT], BF16, tag=f"hT{m}", name=f"hT{m}")
                  for m in range(NC_FF)]

            # ones / zeros columns of the attnV stationary (written once)
            for t_ in range(4):
                for c in range(NC_D):
                    nc.vector.memset(seg_vx[t_][:, c * VXW + 64: c * VXW + 128], 0.0)
                    nc.vector.memset(seg_vx[t_][:, c * VXW + 64: c * VXW + 65], 1.0)

            # weights: double-buffered persistent slots
            wq_t = [[P["wpool"].tile([128, D], BF16, tag=f"wq{b}_{k}", name=f"wq{b}_{k}")
                     for k in range(NC_D)] for b in range(2)]
            wk_t = [[P["wpool"].tile([128, D], BF16, tag=f"wk{b}_{k}", name=f"wk{b}_{k}")
                     for k in range(NC_D)] for b in range(2)]
            wv_t = [[P["wpool"].tile([128, D], BF16, tag=f"wv{b}_{k}", name=f"wv{b}_{k}")
                     for k in range(NC_D)] for b in range(2)]
            wo_t = [[P["wpool"].tile([128, D], BF16, tag=f"wo{b}_{k}", name=f"wo{b}_{k}")
                     for k in range(NC_D)] for b in range(2)]
            w1_t = [[P["wpool"].tile([128, FF], BF16, tag=f"w1{b}_{k}", name=f"w1{b}_{k}")
                     for k in range(NC_D)] for b in range(2)]
            w2_t = [[P["wpool"].tile([128, D], BF16, tag=f"w2{b}_{k}", name=f"w2{b}_{k}")
                     for k in range(NC_FF)] for b in range(2)]
            # input/head weights (persistent, loaded once)
            win_t = P["wpool"].tile([128, D], BF16, tag="win", name="win")
            wp1_t = [P["wpool"].tile([128, D // 2], BF16, tag=f"wp1{k}", name=f"wp1{k}")
                     for k in range(NC_D)]
            wp2_t = [P["wpool"].tile([128, D // 4], BF16, tag=f"wp2{k}", name=f"wp2{k}")
                     for k in range(2)]
            wp3_t = P["wpool"].tile([128, ACT_DIM], BF16, tag="wp3", name="wp3")

            # ---------------- helpers ----------------
            def ap_vec(nm, idx):
                t = vt.get(nm)
                return t[:, idx:idx + 1] if t is not None else None

            def layernorm(chunk_aps, nfeat, g_fn, b_fn, gelu=False):
                nch = len(chunk_aps)
                sums = P["pstat"].tile([1, TT], F32, tag="st")
                sumsq = P["pstat"].tile([1, TT], F32, tag="st")
                for c in range(nch):
                    xc = chunk_aps[c]
                    sqt = P["sq"].tile([128, TT], BF16, tag="sqt")
                    nc.vector.tensor_mul(sqt, xc, xc)
                    nc.tensor.matmul(sums, ones_col, xc,
                                     start=(c == 0), stop=(c == nch - 1))
                    nc.tensor.matmul(sumsq, ones_col, sqt,
                                     start=(c == 0), stop=(c == nch - 1))
                m16 = P["mini"].tile([1, TT], BF16, tag="m16")
                nc.scalar.mul(m16, sums, 1.0 / nfeat)
                msq = P["mini"].tile([1, TT], F32, tag="msq")
                nc.vector.tensor_mul(msq, m16, m16)
                e2 = P["mini"].tile([1, TT], F32, tag="e2")
                # e2 = (sumsq + n*eps) * (1/n) = sumsq/n + eps
                nc.vector.tensor_scalar(out=e2, in0=sumsq, scalar1=float(nfeat) * EPS,
                                        scalar2=1.0 / nfeat, op0=ALU.add, op1=ALU.mult)
                nc.vector.tensor_sub(e2, e2, msq)
                nc.vector.reciprocal_approx_fast(out=msq, in_=e2)
                r16 = P["mini"].tile([1, TT], BF16, tag="r16")
                nc.scalar.sqrt(r16, msq)         # r16 = rstd
                M = P["bcast"].tile([128, TT], BF16, tag="Mb")
                nc.gpsimd.partition_broadcast(M, m16)
                R = P["bcast"].tile([128, TT], BF16, tag="Rb")
                nc.gpsimd.partition_broadcast(R, r16)
                for c in range(nch):
                    xc = chunk_aps[c]
                    g_ap, b_ap = g_fn(c), b_fn(c)
                    nc.vector.tensor_sub(xc, xc, M)
                    if gelu:
                        nc.vector.tensor_mul(xc, xc, R)
                        nc.scalar.activation(xc, xc, AF.Gelu,
                                             bias=b_ap if b_ap is not None else 0.0,
                                             scale=g_ap if g_ap is not None else 1.0)
                    elif g_ap is None and b_ap is None:
                        nc.vector.tensor_mul(xc, xc, R)
                    else:
                        nc.vector.scalar_tensor_tensor(
                            xc, xc, g_ap if g_ap is not None else 1.0, R,
                            ALU.mult, ALU.mult)
                        if b_ap is not None:
                            nc.scalar.activation(xc, xc, AF.Identity, bias=b_ap, scale=1.0)

            def proj_fm(w_tiles, in_aps, out_aps, bias_fn, copy_dve=False,
                        act=None, resid=False, kpart=128):
                """feature-major projection: out[mc] = W.T @ in ( + bias ), psum-wise."""
                n_out = len(out_aps)
                n_in = len(in_aps)
                for mc in range(n_out):
                    ps = P["pbig"].tile([128, TT], F32, tag="pb")
                    for kc in range(n_in):
                        nc.tensor.matmul(
                            ps, w_tiles[kc][0:kpart, mc * 128:(mc + 1) * 128],
                            in_aps[kc][0:kpart, :],
                            start=(kc == 0), stop=(kc == n_in - 1))
                    b_ap = bias_fn(mc) if bias_fn is not None else None
                    if resid:
                        xc = out_aps[mc]
                        nc.vector.scalar_tensor_tensor(
                            xc, ps, b_ap if b_ap is not None else 0.0, xc,
                            ALU.add, ALU.add)
                    elif act == "gelu":
                        nc.scalar.activation(out_aps[mc], ps, AF.Gelu,
                                             bias=b_ap if b_ap is not None else 0.0,
                                             scale=1.0)
                    elif copy_dve and b_ap is None:
                        nc.vector.tensor_copy(out_aps[mc], ps)
                    else:
                        nc.scalar.activation(out_aps[mc], ps, AF.Identity,
                                             bias=b_ap if b_ap is not None else 0.0,
                                             scale=1.0)

            def load_weights(lx, b):
                for k in range(NC_D):
                    nc.sync.dma_start(out=wq_t[b][k], in_=wq_d[lx, k * 128:(k + 1) * 128, :])
                    nc.sync.dma_start(out=wk_t[b][k], in_=wk_d[lx, k * 128:(k + 1) * 128, :])
                    nc.sync.dma_start(out=wv_t[b][k], in_=wv_d[lx, k * 128:(k + 1) * 128, :])
                    nc.sync.dma_start(out=wo_t[b][k], in_=wo_d[lx, k * 128:(k + 1) * 128, :])
                    nc.sync.dma_start(out=w1_t[b][k], in_=w1_d[lx, k * 128:(k + 1) * 128, :])
                for k in range(NC_FF):
                    nc.sync.dma_start(out=w2_t[b][k], in_=w2_d[lx, k * 128:(k + 1) * 128, :])

            def attn_block(seg, lq, b):
                """qkv projections + attention for `seg`; weights from buffer b."""
                xs = [xT[c][seg][:, :] for c in range(NC_D)]
                proj_fm(wq_t[b], xs, [t[:, :] for t in seg_q],
                        (lambda mc: ap_vec("BQ", lq * 4 + mc)) if flags["bq"] else None,
                        copy_dve=True)
                proj_fm(wk_t[b], xs, [t[:, :] for t in seg_k],
                        (lambda mc: ap_vec("BK", lq * 4 + mc)) if flags["bk"] else None,
                        copy_dve=True)
                # v projection (token-major) -> seg_vx strided layout
                for ts in range(4):
                    vp = P["pbig"].tile([128, D], F32, tag="pb")
                    for kc in range(NC_D):
                        nc.tensor.matmul(
                            vp, xT[kc][seg][:, ts * 128:(ts + 1) * 128],
                            wv_t[b][kc],
                            start=(kc == 0), stop=(kc == NC_D - 1) and not flags["bv"])
                    if flags["bv"]:
                        nc.tensor.matmul(vp, ones[0:1, 0:128],
                                         vt["BV"][:, lq * D:(lq + 1) * D],
                                         start=False, stop=True)
                    vx = seg_vx[ts]
                    # even heads -> pair base +0 ; odd heads -> pair base +128
                    nc.vector.tensor_copy(
                        vx.rearrange("p (c w) -> p c w", w=VXW)[:, :, 0:64],
                        vp.rearrange("p (c w) -> p c w", w=128)[:, :, 0:64])
                    nc.vector.tensor_copy(
                        vx.rearrange("p (c w) -> p c w", w=VXW)[:, :, 128:192],
                        vp.rearrange("p (c w) -> p c w", w=128)[:, :, 64:128])
                # attention: 2 batches x 4 head-pairs
                for b2 in range(2):
                    bcol = b2 * S
                    for c in range(NC_D):  # head pair (2c, 2c+1) lives in chunk c
                        scp = [P["pbig"].tile([128, 2 * S], F32, tag="pb", name=f"scp{hh}")
                               for hh in range(2)]
                        # 4 score matmuls back-to-back; hh=0 uses PE rows 0:64,
                        # hh=1 rows 64:128 -> they overlap in the array
                        for kc in range(2):
                            for hh in range(2):
                                roff = hh * HD
                                nc.tensor.matmul(
                                    scp[hh][:, kc * S:(kc + 1) * S],
                                    seg_k[c][roff:roff + HD,
                                             bcol + kc * 128: bcol + (kc + 1) * 128],
                                    seg_q[c][roff:roff + HD, bcol:bcol + S],
                                    start=True, stop=True)
                        es2 = []
                        for hh in range(2):
                            esh = P["exps"].tile([128, 2 * S], BF16, tag="es",
                                                 name=f"es{hh}")
                            nc.scalar.activation(esh, scp[hh], AF.Exp, bias=0.0,
                                                 scale=scale)
                            es2.append(esh)
                        # attnV with folded denominator (both heads share one
                        # psum bank, disjoint column halves):
                        # even head: [V_e | ones | 0] -> O in rows 0:64, denom row 64
                        # odd head:  [ones | 0 | V_o] -> denom row 0, O in rows 64:128
                        otp = P["pbig"].tile([128, 2 * S], F32, tag="pb", name="otp")
                        for hh in range(2):
                            off = c * VXW + hh * 64
                            for kc in range(2):
                                nc.tensor.matmul(
                                    otp[:, hh * S:(hh + 1) * S],
                                    seg_vx[b2 * 2 + kc][:, off:off + 128],
                                    es2[hh][:, kc * S:(kc + 1) * S],
                                    start=(kc == 0), stop=(kc == 1),
                                    skip_group_check=True)
                        # denominators stay in their own partitions (even
                        # head's at p64, odd head's at p0 -- engines cannot
                        # shift partitions); the selector matmul broadcasts
                        # both rows across partitions, then one reciprocal on
                        # the partition-0-based psum tile (custom DVE ops only
                        # work from partition 0).
                        denf = denf_t[(b2 * NC_D + c) % 2]
                        nc.vector.tensor_copy(denf[64:65, :], otp[64:65, 0:S])
                        nc.vector.tensor_copy(denf[0:1, :], otp[0:1, S:2 * S])
                        db2 = P["pstat"].tile([128, S], F32, tag="rb2", bufs=1)
                        nc.tensor.matmul(db2, selm, denf, start=True, stop=True)
                        rbsf = P["rbs"].tile([128, S], F32, tag="rbs")
                        nc.vector.reciprocal_approx_fast(out=rbsf, in_=db2)
                        nc.vector.tensor_mul(
                            seg_o[c][0:HD, bcol:bcol + S], otp[0:HD, 0:S], rbsf[0:HD, :])
                        nc.vector.tensor_mul(
                            seg_o[c][HD:128, bcol:bcol + S], otp[HD:128, S:2 * S],
                            rbsf[HD:128, :])

            def wo_proj(seg, l_, b):
                xs = [xT[c][seg][:, :] for c in range(NC_D)]
                proj_fm(wo_t[b], [t[:, :] for t in seg_o], xs,
                        (lambda mc: ap_vec("BO", l_ * 4 + mc)) if flags["bo"] else None,
                        resid=True)

            def ffn(seg, l_, b):
                xs = [xT[c][seg][:, :] for c in range(NC_D)]
                for mc in range(NC_FF):
                    ps = P["pbig"].tile([128, TT], F32, tag="pb")
                    for kc in range(NC_D):
                        nc.tensor.matmul(
                            ps, w1_t[b][kc][:, mc * 128:(mc + 1) * 128], xs[kc],
                            start=(kc == 0), stop=(kc == NC_D - 1))
                    nc.scalar.activation(
                        hT[mc][:, :], ps, AF.Gelu,
                        bias=ap_vec("B1", l_ * 16 + mc) if flags["b1"] else 0.0,
                        scale=1.0)
                for mcD in range(NC_D):
                    ps = P["pbig"].tile([128, TT], F32, tag="pb")
                    for kc in range(NC_FF):
                        nc.tensor.matmul(
                            ps, w2_t[b][kc][:, mcD * 128:(mcD + 1) * 128],
                            hT[kc][:, :],
                            start=(kc == 0), stop=(kc == NC_FF - 1))
                    nc.vector.scalar_tensor_tensor(
                        xs[mcD], ps,
                        ap_vec("B2", l_ * 4 + mcD) if flags["b2"] else 0.0,
                        xs[mcD], ALU.add, ALU.add)

            def ln1(seg, l_):
                xs = [xT[c][seg][:, :] for c in range(NC_D)]
                layernorm(xs, D,
                          (lambda c: ap_vec("G1", l_ * 4 + c)) if flags["g1"] else (lambda c: None),
                          (lambda c: ap_vec("BE1", l_ * 4 + c)) if flags["be1"] else (lambda c: None))

            def ln2(seg, l_):
                xs = [xT[c][seg][:, :] for c in range(NC_D)]
                layernorm(xs, D,
                          (lambda c: ap_vec("G2", l_ * 4 + c)) if flags["g2"] else (lambda c: None),
                          (lambda c: ap_vec("BE2", l_ * 4 + c)) if flags["be2"] else (lambda c: None))

            # ---------------- input stage ----------------
            load_weights(0, 0)
            nc.sync.dma_start(out=win_t[0:OBS, :], in_=win_d[:, :])
            obs_flat = obs_d.rearrange("b s f -> (b s) f")
            for seg in range(NSEG):
                obsT = seg_k[0]  # [96, 512] region used
                for ts in range(4):
                    t0 = seg * TT + ts * 128
                    ot = P["scratch"].tile([128, OBS], BF16, tag="obs_in")
                    nc.sync.dma_start(out=ot, in_=obs_flat[t0:t0 + 128, :])
                    tp = P["ptp"].tile([128, 128], BF16, tag="tpb")
                    nc.tensor.transpose(tp[0:OBS, :], ot, ident)
                    nc.scalar.copy(obsT[0:OBS, ts * 128:(ts + 1) * 128], tp[0:OBS, :])
                xs = [xT[c][seg][:, :] for c in range(NC_D)]
                proj_fm([win_t], [obsT[:, :]], xs,
                        (lambda mc: ap_vec("BIN", mc)) if flags["bin_"] else None,
                        kpart=OBS)
                layernorm(xs, D,
                          (lambda c: ap_vec("GIN", c)) if flags["gin"] else (lambda c: None),
                          (lambda c: ap_vec("BEIN", c)) if flags["bein"] else (lambda c: None),
                          gelu=True)
                for c in range(NC_D):
                    xc = xs[c]
                    nc.vector.tensor_add(
                        xc.rearrange("p (b s) -> p b s", s=S),
                        xc.rearrange("p (b s) -> p b s", s=S),
                        peT[:, c * S:(c + 1) * S].unsqueeze(1).broadcast_to([128, TT // S, S]))

            # ---------------- layers ----------------
            # steady-state emission per seg s:
            #   [attn(s+1)] [LN1(s)] [FFN(s)] [Wo(s+1)] [LN2(s)]
            # with attn/Wo rolling into the next layer at s==3.
            if dbg_att and n_layers > 0:
                attn_block(0, 0, 0)
                for c in range(NC_D):
                    nc.sync.dma_start(out=adbg_d[0, c * 128:(c + 1) * 128, :],
                                      in_=seg_q[c][:, :])
                    nc.sync.dma_start(out=adbg_d[1, c * 128:(c + 1) * 128, :],
                                      in_=seg_k[c][:, :])
                    nc.sync.dma_start(out=adbg_d[2, c * 128:(c + 1) * 128, :],
                                      in_=seg_o[c][:, :])
                for t_ in range(4):
                    nc.sync.dma_start(out=vdbg_d[t_, :, :], in_=seg_vx[t_][:, :])
                for i in range(2):
                    nc.sync.dma_start(out=rdbg_d[i, :, :], in_=denf_t[i][:, :])
            elif n_layers > 0:
                attn_block(0, 0, 0)
                wo_proj(0, 0, 0)
            for l in range(n_layers if not dbg_att else 0):
                b = l % 2
                if l + 1 < n_layers:
                    load_weights(l + 1, 1 - b)
                for s in range(NSEG):
                    if s + 1 < NSEG:
                        attn_block(s + 1, l, b)
                    elif l + 1 < n_layers:
                        attn_block(0, l + 1, 1 - b)
                    ln1(s, l)
                    ffn(s, l, b)
                    if s + 1 < NSEG:
                        wo_proj(s + 1, l, b)
                    elif l + 1 < n_layers:
                        wo_proj(0, l + 1, 1 - b)
                    ln2(s, l)

            if dbg_x:
                for c in range(NC_D):
                    for s in range(NSEG):
                        nc.sync.dma_start(
                            out=xdbg_d[c * 128:(c + 1) * 128, s * TT:(s + 1) * TT],
                            in_=xT[c][s][:, :])

            # ---------------- head (phase-major, 4-seg pipeline) ----------------
            if emit_head:
                for k in range(NC_D):
                    nc.sync.dma_start(out=wp1_t[k], in_=wp1_d[k * 128:(k + 1) * 128, :])
                for k in range(2):
                    nc.sync.dma_start(out=wp2_t[k], in_=wp2_d[k * 128:(k + 1) * 128, :])
                nc.sync.dma_start(out=wp3_t[0:128, :], in_=wp3_d[:, :])
                y1s = [[seg_q[s][:, :], seg_k[s][:, :]] for s in range(NSEG)]
                y2s = [seg_vx[s][:, 0:TT] for s in range(NSEG)]
                for s in range(NSEG):
                    xs = [xT[c][s][:, :] for c in range(NC_D)]
                    proj_fm(wp1_t, xs, y1s[s],
                            (lambda mc: ap_vec("BP1", mc)) if flags["bp1"] else None)
                    layernorm(y1s[s], D // 2,
                              (lambda c: ap_vec("GP1", c)) if flags["gp1"] else (lambda c: None),
                              (lambda c: ap_vec("BEP1", c)) if flags["bep1"] else (lambda c: None),
                              gelu=True)
                for s in range(NSEG):
                    proj_fm(wp2_t, y1s[s], [y2s[s]],
                            (lambda mc: ap_vec("BP2", 0)) if flags["bp2"] else None)
                    layernorm([y2s[s]], D // 4,
                              (lambda c: ap_vec("GP2", 0)) if flags["gp2"] else (lambda c: None),
                              (lambda c: ap_vec("BEP2", 0)) if flags["bep2"] else (lambda c: None),
                              gelu=True)
                for s in range(NSEG):
                    actp = P["pbig"].tile([ACT_DIM, TT], F32, tag="pb")
                    nc.tensor.matmul(actp, wp3_t[0:128, :], y2s[s], start=True, stop=True)
                    actT = P["headt"].tile([ACT_DIM, TT], BF16, tag="actT")
                    nc.scalar.activation(actT[0:ACT_DIM, :], actp, AF.Tanh,
                                         bias=vt["BP3"][0:ACT_DIM, 0:1] if flags["bp3"] else 0.0,
                                         scale=1.0)
                    if flags["asc"] or flags["abi"]:
                        nc.scalar.activation(
                            actT[0:ACT_DIM, :], actT[0:ACT_DIM, :], AF.Identity,
                            bias=vt["ABI"][0:ACT_DIM, 0:1] if flags["abi"] else 0.0,
                            scale=vt["ASC"][0:ACT_DIM, 0:1] if flags["asc"] else 1.0)
                    for ts in range(4):
                        tp = P["ptp"].tile([128, 128], BF16, tag="tpb")
                        nc.tensor.transpose(tp[0:128, 0:ACT_DIM],
                                            actT[0:ACT_DIM, ts * 128:(ts + 1) * 128],
                                            ident[0:ACT_DIM, 0:ACT_DIM])
                        ob = P["scratch"].tile([128, ACT_DIM], F32, tag="ob")
                        nc.vector.tensor_copy(ob, tp[0:128, 0:ACT_DIM])
                        nc.sync.dma_start(
                            out=out_d[s * TT + ts * 128: s * TT + (ts + 1) * 128, :],
                            in_=ob)

    nc.compile()
    return nc, extra


# ======================================================================
# Self-contained kernel entry point: takes FULL inputs, shards batch over
# 8 NeuronCores (data-parallel), runs the Bass kernel, gathers output.
# ======================================================================
from concourse.bass_utils import run_bass_kernel_spmd

N_CORES = 8


def make_in_maps(inputs, extra):
    base = dict(extra)
    for k in ["W_in", "Wq", "Wk", "Wv", "Wo", "W1", "W2", "Wp1", "Wp2", "Wp3"]:
        base[k] = np.ascontiguousarray(np.asarray(inputs[k], np.float32).astype(BF))
    obs = np.asarray(inputs["observations"], np.float32).astype(BF)
    n_b = obs.shape[0]
    per = n_b // N_CORES
    in_maps = []
    for c in range(N_CORES):
        m = dict(base)
        m["observations"] = np.ascontiguousarray(obs[c * per:(c + 1) * per])
        in_maps.append(m)
    return in_maps, per


def kernel(**inputs):
    inputs = {k: np.asarray(v) for k, v in inputs.items()}
    nc, extra = build(inputs, n_layers=8, emit_head=True, dbg_x=False)
    in_maps, per = make_in_maps(inputs, extra)

    last_err = None
    for attempt in range(4):
        try:
            res = run_bass_kernel_spmd(nc, in_maps, core_ids=list(range(N_CORES)),
                                       trace=False)
            outs = [res.results[c]["OUT"].reshape(per, S, ACT_DIM)
                    for c in range(N_CORES)]
            return np.concatenate(outs, axis=0)
        except Exception as e:  # transient NRT_EXEC_UNIT_UNRECOVERABLE etc.
            last_err = e
            import time as _time
            _time.sleep(3.0 * (attempt + 1))
    raise last_err


# revision 32
# speedup vs baseline: 1.1773x; 1.0909x over previous
"""Transformer policy kernel builder for TRN2 (Bass/Tile), feature-major, bf16.

Per core (data-parallel over batch): BC=8 batches x S=256 -> T=2048 tokens.
D=512 (4 chunks), H=8 heads (HD=64), FF=2048 (16 chunks), L layers.

v2 design vs v1 baseline:
  - bf16 weights + activations (psum stays f32): halves DVE time, weight DMA,
    and SBUF pressure.
  - softmax denominator folded into the attnV stationary via a
    [V_even | ones | zeros(63) | V_odd] 192-col per-head-pair layout: the even
    head's matmul yields O_even in psum rows 0:64 and the denominator in row
    64; the odd head's yields the denominator in row 0 and O_odd in rows
    64:128 -- no separate row-sum matmuls and no SBUF->SBUF partition-shift
    DMA for the odd head.
  - scores matmuls for a head pair emitted back-to-back: they target PE row
    groups (0,0)/(64,0) and can overlap in the array.
  - double-buffered layer weights (bf16 makes them fit): layer l+1 weights DMA
    while layer l computes.
  - single-wave FFN (16 hidden tiles) with per-output-chunk sequential W2
    accumulation: psum pressure low, next-seg Wo projection emitted before
    LN2 so the PE never drains while the LN vector chain runs.
  - head emitted phase-major (all Wp1+LN, all Wp2+LN, all Wp3+tanh) with
    per-segment buffers so the 4 segments pipeline.
"""
import math
import contextlib
import numpy as np
import ml_dtypes

import concourse.bass as bass
import concourse.bacc as bacc
import concourse.tile as tile
from concourse import mybir

F32 = mybir.dt.float32
F32R = mybir.dt.float32r
BF16 = mybir.dt.bfloat16
AF = mybir.ActivationFunctionType
ALU = mybir.AluOpType

BC = 8
S = 256
T = BC * S
OBS = 96
ACT_DIM = 29
D = 512
H = 8
HD = 64
FF = 2048
NC_D = D // 128
NC_FF = FF // 128
TT = 512
NSEG = T // TT
EPS = 1e-5
L_MAX = 8
VXW = 192          # per head-pair stride in the attnV stationary layout
BF = ml_dtypes.bfloat16


def _nz(a):
    return a is not None and bool(np.any(np.asarray(a) != 0))


def _ng(a):
    return a is not None and bool(np.any(np.asarray(a) != 1))


def build(inputs, n_layers=8, emit_head=True, dbg_x=False, dbg_att=False):
    """inputs: dict of full np arrays (reference naming). Returns (nc, extra_in_map)."""
    nc = bacc.Bacc("TRN2", target_bir_lowering=False, debug=False)

    flags = dict(
        bin_=_nz(inputs["b_in"]), gin=_ng(inputs["g_in"]), bein=_nz(inputs["be_in"]),
        bq=_nz(inputs["bq"]), bk=_nz(inputs["bk"]), bv=_nz(inputs["bv"]), bo=_nz(inputs["bo"]),
        g1=_ng(inputs["g1"]), be1=_nz(inputs["be1"]), b1=_nz(inputs["b1"]), b2=_nz(inputs["b2"]),
        g2=_ng(inputs["g2"]), be2=_nz(inputs["be2"]),
        bp1=_nz(inputs["bp1"]), gp1=_ng(inputs["gp1"]), bep1=_nz(inputs["bep1"]),
        bp2=_nz(inputs["bp2"]), gp2=_ng(inputs["gp2"]), bep2=_nz(inputs["bep2"]),
        bp3=_nz(inputs["bp3"]), asc=_ng(inputs["action_scale"]), abi=_nz(inputs["action_bias"]),
    )

    def din(name, shape, dt=BF16):
        return nc.dram_tensor(name, shape, dt, kind="ExternalInput").ap()

    obs_d = din("observations", (BC, S, OBS))
    win_d = din("W_in", (OBS, D))
    wq_d = din("Wq", (L_MAX, D, D)); wk_d = din("Wk", (L_MAX, D, D))
    wv_d = din("Wv", (L_MAX, D, D)); wo_d = din("Wo", (L_MAX, D, D))
    w1_d = din("W1", (L_MAX, D, FF)); w2_d = din("W2", (L_MAX, FF, D))
    wp1_d = din("Wp1", (D, D // 2)); wp2_d = din("Wp2", (D // 2, D // 4))
    wp3_d = din("Wp3", (D // 4, ACT_DIM))
    ident_d = din("IDENT", (128, 128))
    ones_d = din("ONES", (128, 128))
    pet_d = din("PET", (D, S))
    out_d = nc.dram_tensor("OUT", (T, ACT_DIM), F32, kind="ExternalOutput").ap()
    if dbg_x:
        xdbg_d = nc.dram_tensor("XDBG", (D, T), BF16, kind="ExternalOutput").ap()
    if dbg_att:
        adbg_d = nc.dram_tensor("ADBG", (3, D, TT), BF16, kind="ExternalOutput").ap()
        vdbg_d = nc.dram_tensor("VDBG", (4, 128, NC_D * VXW), BF16, kind="ExternalOutput").ap()
        rdbg_d = nc.dram_tensor("RDBG", (2, 128, S), BF16, kind="ExternalOutput").ap()

    extra = {
        "IDENT": np.eye(128, dtype=BF),
        "ONES": np.ones((128, 128), BF),
    }
    # selector for broadcasting softmax reciprocals: out rows 0:64 get rec
    # row 64 (even head), rows 64:128 get rec row 0 (odd head)
    sel = np.zeros((128, 128), np.float32)
    sel[64, 0:64] = 1.0
    sel[0, 64:128] = 1.0
    extra["SELM"] = sel.astype(BF)
    selm_d = din("SELM", (128, 128), BF16)
    pos = np.arange(S, dtype=np.float32)[:, None]
    div = np.exp(np.arange(0, D, 2, dtype=np.float32) * (-math.log(10000.0) / D))
    pe = np.zeros((S, D), dtype=np.float32)
    pe[:, 0::2] = np.sin(pos * div)
    pe[:, 1::2] = np.cos(pos * div)
    extra["PET"] = np.ascontiguousarray(pe.T).astype(BF)

    def vec_tensor(name, arr):
        a = np.asarray(arr, np.float32).reshape(-1)
        n = a.size // 128
        extra[name] = np.ascontiguousarray(a.reshape(n, 128).T)
        return din(name, (128, n), F32)

    dv = {}
    for key, nm in [("bq", "BQ"), ("bk", "BK"), ("bo", "BO"), ("b1", "B1"), ("b2", "B2"),
                    ("b_in", "BIN"), ("g_in", "GIN"), ("be_in", "BEIN"),
                    ("g1", "G1"), ("be1", "BE1"), ("g2", "G2"), ("be2", "BE2"),
                    ("bp1", "BP1"), ("gp1", "GP1"), ("bep1", "BEP1"),
                    ("bp2", "BP2"), ("gp2", "GP2"), ("bep2", "BEP2")]:
        fkey = {"b_in": "bin_", "g_in": "gin", "be_in": "bein"}.get(key, key)
        if flags[fkey]:
            dv[nm] = vec_tensor(nm + "v", inputs[key])
    if flags["bv"]:
        extra["BVr"] = np.asarray(inputs["bv"], np.float32).astype(BF).reshape(L_MAX, D)
        dv["BV"] = din("BVr", (L_MAX, D))

    def vec29(name, arr):
        a = np.zeros((128, 1), np.float32)
        a[:ACT_DIM, 0] = np.asarray(arr, np.float32).reshape(-1)
        extra[name] = a
        return din(name, (128, 1), F32)
    if flags["bp3"]:
        dv["BP3"] = vec29("BP3v", inputs["bp3"])
    if flags["asc"]:
        dv["ASC"] = vec29("ASCv", inputs["action_scale"])
    if flags["abi"]:
        dv["ABI"] = vec29("ABIv", inputs["action_bias"])

    scale = 1.0 / math.sqrt(HD)

    with tile.TileContext(nc) as tc:
        with contextlib.ExitStack() as ctx:
            P = {}
            P["persist"] = ctx.enter_context(tc.tile_pool(name="persist", bufs=1))
            P["wpool"] = ctx.enter_context(tc.tile_pool(name="wpool", bufs=1))
            P["xpool"] = ctx.enter_context(tc.tile_pool(name="xpool", bufs=1))
            P["segt"] = ctx.enter_context(tc.tile_pool(name="segt", bufs=1))
            P["exps"] = ctx.enter_context(tc.tile_pool(name="exps", bufs=4))
            P["sq"] = ctx.enter_context(tc.tile_pool(name="sq", bufs=2))
            P["scratch"] = ctx.enter_context(tc.tile_pool(name="scratch", bufs=4))
            P["mini"] = ctx.enter_context(tc.tile_pool(name="mini", bufs=2))
            P["bcast"] = ctx.enter_context(tc.tile_pool(name="bcast", bufs=2))
            P["rbs"] = ctx.enter_context(tc.tile_pool(name="rbs", bufs=4))
            P["rec"] = ctx.enter_context(tc.tile_pool(name="rec", bufs=2))
            P["hpool"] = ctx.enter_context(tc.tile_pool(name="hpool", bufs=1))
            P["headt"] = ctx.enter_context(tc.tile_pool(name="headt", bufs=4))
            # PSUM is bank-granular (8 x 2KB/partition):
            # pb 5x[128,512]f32 (5) + st 2x[1,512]f32 (2) + tpb 1 (1) = 8 banks
            P["pbig"] = ctx.enter_context(tc.tile_pool(name="pbig", bufs=4, space="PSUM"))
            P["ptp"] = ctx.enter_context(tc.tile_pool(name="ptp", bufs=1, space="PSUM"))
            P["pstat"] = ctx.enter_context(tc.tile_pool(name="pstat", bufs=2, space="PSUM"))

            # ---------------- constants ----------------
            ident = P["persist"].tile([128, 128], BF16, tag="ident")
            nc.sync.dma_start(out=ident, in_=ident_d[:, :])
            ones_w = 128 if flags["bv"] else 8
            ones = P["persist"].tile([128, ones_w], BF16, tag="ones")
            nc.sync.dma_start(out=ones, in_=ones_d[:, 0:ones_w])
            ones_col = ones[:, 0:1]
            selm = P["persist"].tile([128, 128], BF16, tag="selm")
            nc.sync.dma_start(out=selm, in_=selm_d[:, :])
            # denominator staging tiles: only partitions 0 and 64 are ever
            # written; the rest must be zero so the selector matmul contracts
            # cleanly
            denf_t = [P["persist"].tile([128, S], BF16, tag=f"denf{i}", name=f"denf{i}")
                      for i in range(2)]
            for i in range(2):
                nc.vector.memset(denf_t[i], 0.0)

            peT = P["persist"].tile([128, NC_D * S], BF16, tag="peT")
            for c in range(NC_D):
                nc.sync.dma_start(out=peT[:, c * S:(c + 1) * S],
                                  in_=pet_d[c * 128:(c + 1) * 128, :])

            vt = {}
            for nm, d in dv.items():
                if nm == "BV":
                    t = P["persist"].tile([1, L_MAX * D], BF16, tag="c_BV")
                    for l in range(L_MAX):
                        nc.sync.dma_start(out=t[:, l * D:(l + 1) * D], in_=d[l:l + 1, :])
                else:
                    t = P["persist"].tile([128, d.shape[1]], F32, tag=f"c_{nm}")
                    nc.sync.dma_start(out=t, in_=d[:, :])
                vt[nm] = t

            # ---------------- big tiles ----------------
            xT = [[P["xpool"].tile([128, TT], BF16, tag=f"xT{c}_{s}", name=f"xT{c}_{s}")
                   for s in range(NSEG)] for c in range(NC_D)]
            seg_q = [P["segt"].tile([128, TT], BF16, tag=f"sq{c}", name=f"sq{c}") for c in range(NC_D)]
            seg_k = [P["segt"].tile([128, TT], BF16, tag=f"sk{c}", name=f"sk{c}") for c in range(NC_D)]
            seg_o = [P["segt"].tile([128, TT], BF16, tag=f"so{c}", name=f"so{c}") for c in range(NC_D)]
            # attnV stationary: 4 ts-tiles x [128 tok, 4 pairs x 192]
            seg_vx = [P["segt"].tile([128, NC_D * VXW], BF16, tag=f"svx{t_}", name=f"svx{t_}")
                      for t_ in range(4)]
            hT = [P["hpool"].tile([128, TT], BF16, tag=f"hT{m}", name=f"hT{m}")
                  for m in range(NC_FF)]

            # ones / zeros columns of the attnV stationary (written once)
            for t_ in range(4):
                for c in range(NC_D):
                    nc.vector.memset(seg_vx[t_][:, c * VXW + 64: c * VXW + 128], 0.0)
                    nc.vector.memset(seg_vx[t_][:, c * VXW + 64: c * VXW + 65], 1.0)

            # weights: double-buffered persistent slots
            wq_t = [[P["wpool"].tile([128, D], BF16, tag=f"wq{b}_{k}", name=f"wq{b}_{k}")
                     for k in range(NC_D)] for b in range(2)]
            wk_t = [[P["wpool"].tile([128, D], BF16, tag=f"wk{b}_{k}", name=f"wk{b}_{k}")
                     for k in range(NC_D)] for b in range(2)]
            wv_t = [[P["wpool"].tile([128, D], BF16, tag=f"wv{b}_{k}", name=f"wv{b}_{k}")
                     for k in range(NC_D)] for b in range(2)]
            wo_t = [[P["wpool"].tile([128, D], BF16, tag=f"wo{b}_{k}", name=f"wo{b}_{k}")
                     for k in range(NC_D)] for b in range(2)]
            w1_t = [[P["wpool"].tile([128, FF], BF16, tag=f"w1{b}_{k}", name=f"w1{b}_{k}")
                     for k in range(NC_D)] for b in range(2)]
            w2_t = [[P["wpool"].tile([128, D], BF16, tag=f"w2{b}_{k}", name=f"w2{b}_{k}")
                     for k in range(NC_FF)] for b in range(2)]
            # input/head weights (persistent, loaded once)
            win_t = P["wpool"].tile([128, D], BF16, tag="win", name="win")
            wp1_t = [P["wpool"].tile([128, D // 2], BF16, tag=f"wp1{k}", name=f"wp1{k}")
                     for k in range(NC_D)]
            wp2_t = [P["wpool"].tile([128, D // 4], BF16, tag=f"wp2{k}", name=f"wp2{k}")
                     for k in range(2)]
            wp3_t = P["wpool"].tile([128, ACT_DIM], BF16, tag="wp3", name="wp3")

            # ---------------- helpers ----------------
            def ap_vec(nm, idx):
                t = vt.get(nm)
                return t[:, idx:idx + 1] if t is not None else None

            def layernorm(chunk_aps, nfeat, g_fn, b_fn, gelu=False):
                nch = len(chunk_aps)
                sums = P["pstat"].tile([1, TT], F32, tag="st")
                sumsq = P["pstat"].tile([1, TT], F32, tag="st")
                for c in range(nch):
                    xc = chunk_aps[c]
                    sqt = P["sq"].tile([128, TT], BF16, tag="sqt")
                    nc.vector.tensor_mul(sqt, xc, xc)
                    nc.tensor.matmul(sums, ones_col, xc,
                                     start=(c == 0), stop=(c == nch - 1))
                    nc.tensor.matmul(sumsq, ones_col, sqt,
                                     start=(c == 0), stop=(c == nch - 1))
                m16 = P["mini"].tile([1, TT], BF16, tag="m16")
                nc.scalar.mul(m16, sums, 1.0 / nfeat)
                msq = P["mini"].tile([1, TT], F32, tag="msq")
                nc.vector.tensor_mul(msq, m16, m16)
                e2 = P["mini"].tile([1, TT], F32, tag="e2")
                # e2 = (sumsq + n*eps) * (1/n) = sumsq/n + eps
                nc.vector.tensor_scalar(out=e2, in0=sumsq, scalar1=float(nfeat) * EPS,
                                        scalar2=1.0 / nfeat, op0=ALU.add, op1=ALU.mult)
                nc.vector.tensor_sub(e2, e2, msq)
                nc.vector.reciprocal_approx_fast(out=msq, in_=e2)
                r16 = P["mini"].tile([1, TT], BF16, tag="r16")
                nc.scalar.sqrt(r16, msq)         # r16 = rstd
                M = P["bcast"].tile([128, TT], BF16, tag="Mb")
                nc.gpsimd.partition_broadcast(M, m16)
                R = P["bcast"].tile([128, TT], BF16, tag="Rb")
                nc.gpsimd.partition_broadcast(R, r16)
                for c in range(nch):
                    xc = chunk_aps[c]
                    g_ap, b_ap = g_fn(c), b_fn(c)
                    nc.vector.tensor_sub(xc, xc, M)
                    if gelu:
                        nc.vector.tensor_mul(xc, xc, R)
                        nc.scalar.activation(xc, xc, AF.Gelu,
                                             bias=b_ap if b_ap is not None else 0.0,
                                             scale=g_ap if g_ap is not None else 1.0)
                    elif g_ap is None and b_ap is None:
                        nc.vector.tensor_mul(xc, xc, R)
                    else:
                        nc.vector.scalar_tensor_tensor(
                            xc, xc, g_ap if g_ap is not None else 1.0, R,
                            ALU.mult, ALU.mult)
                        if b_ap is not None:
                            nc.scalar.activation(xc, xc, AF.Identity, bias=b_ap, scale=1.0)

            def proj_fm(w_tiles, in_aps, out_aps, bias_fn, copy_dve=False,
                        act=None, resid=False, kpart=128):
                """feature-major projection: out[mc] = W.T @ in ( + bias ), psum-wise."""
                n_out = len(out_aps)
                n_in = len(in_aps)
                for mc in range(n_out):
                    ps = P["pbig"].tile([128, TT], F32, tag="pb")
                    for kc in range(n_in):
                        nc.tensor.matmul(
                            ps, w_tiles[kc][0:kpart, mc * 128:(mc + 1) * 128],
                            in_aps[kc][0:kpart, :],
                            start=(kc == 0), stop=(kc == n_in - 1))
                    b_ap = bias_fn(mc) if bias_fn is not None else None
                    if resid:
                        xc = out_aps[mc]
                        nc.vector.scalar_tensor_tensor(
                            xc, ps, b_ap if b_ap is not None else 0.0, xc,
                            ALU.add, ALU.add)
                    elif act == "gelu":
                        nc.scalar.activation(out_aps[mc], ps, AF.Gelu,
                                             bias=b_ap if b_ap is not None else 0.0,
                                             scale=1.0)
                    elif copy_dve and b_ap is None:
                        nc.scalar.copy(out_aps[mc], ps)
                    else:
                        nc.scalar.activation(out_aps[mc], ps, AF.Identity,
                                             bias=b_ap if b_ap is not None else 0.0,
                                             scale=1.0)

            def load_weights(lx, b):
                for k in range(NC_D):
                    nc.sync.dma_start(out=wq_t[b][k], in_=wq_d[lx, k * 128:(k + 1) * 128, :])
                    nc.sync.dma_start(out=wk_t[b][k], in_=wk_d[lx, k * 128:(k + 1) * 128, :])
                    nc.sync.dma_start(out=wv_t[b][k], in_=wv_d[lx, k * 128:(k + 1) * 128, :])
                    nc.sync.dma_start(out=wo_t[b][k], in_=wo_d[lx, k * 128:(k + 1) * 128, :])
                    nc.sync.dma_start(out=w1_t[b][k], in_=w1_d[lx, k * 128:(k + 1) * 128, :])
                for k in range(NC_FF):
                    nc.sync.dma_start(out=w2_t[b][k], in_=w2_d[lx, k * 128:(k + 1) * 128, :])

            def attn_block(seg, lq, b):
                """qkv projections + attention for `seg`; weights from buffer b."""
                xs = [xT[c][seg][:, :] for c in range(NC_D)]
                proj_fm(wq_t[b], xs, [t[:, :] for t in seg_q],
                        (lambda mc: ap_vec("BQ", lq * 4 + mc)) if flags["bq"] else None,
                        copy_dve=True)
                proj_fm(wk_t[b], xs, [t[:, :] for t in seg_k],
                        (lambda mc: ap_vec("BK", lq * 4 + mc)) if flags["bk"] else None,
                        copy_dve=True)
                # v projection (token-major) -> seg_vx strided layout
                for ts in range(4):
                    vp = P["pbig"].tile([128, D], F32, tag="pb")
                    for kc in range(NC_D):
                        nc.tensor.matmul(
                            vp, xT[kc][seg][:, ts * 128:(ts + 1) * 128],
                            wv_t[b][kc],
                            start=(kc == 0), stop=(kc == NC_D - 1) and not flags["bv"])
                    if flags["bv"]:
                        nc.tensor.matmul(vp, ones[0:1, 0:128],
                                         vt["BV"][:, lq * D:(lq + 1) * D],
                                         start=False, stop=True)
                    vx = seg_vx[ts]
                    # even heads -> pair base +0 ; odd heads -> pair base +128
                    # (scalar engine: Copy is tableless, keeps DVE free)
                    nc.scalar.copy(
                        vx.rearrange("p (c w) -> p c w", w=VXW)[:, :, 0:64],
                        vp.rearrange("p (c w) -> p c w", w=128)[:, :, 0:64])
                    nc.scalar.copy(
                        vx.rearrange("p (c w) -> p c w", w=VXW)[:, :, 128:192],
                        vp.rearrange("p (c w) -> p c w", w=128)[:, :, 64:128])
                # attention: 2 batches x 4 head-pairs
                for b2 in range(2):
                    bcol = b2 * S
                    for c in range(NC_D):  # head pair (2c, 2c+1) lives in chunk c
                        scp = [P["pbig"].tile([128, 2 * S], F32, tag="pb", name=f"scp{hh}")
                               for hh in range(2)]
                        # 4 score matmuls back-to-back; hh=0 uses PE rows 0:64,
                        # hh=1 rows 64:128 -> they overlap in the array
                        for kc in range(2):
                            for hh in range(2):
                                roff = hh * HD
                                nc.tensor.matmul(
                                    scp[hh][:, kc * S:(kc + 1) * S],
                                    seg_k[c][roff:roff + HD,
                                             bcol + kc * 128: bcol + (kc + 1) * 128],
                                    seg_q[c][roff:roff + HD, bcol:bcol + S],
                                    start=True, stop=True)
                        es2 = []
                        for hh in range(2):
                            esh = P["exps"].tile([128, 2 * S], BF16, tag="es",
                                                 name=f"es{hh}")
                            nc.scalar.activation(esh, scp[hh], AF.Exp, bias=0.0,
                                                 scale=scale)
                            es2.append(esh)
                        # attnV with folded denominator (both heads share one
                        # psum bank, disjoint column halves):
                        # even head: [V_e | ones | 0] -> O in rows 0:64, denom row 64
                        # odd head:  [ones | 0 | V_o] -> denom row 0, O in rows 64:128
                        otp = P["pbig"].tile([128, 2 * S], F32, tag="pb", name="otp")
                        for hh in range(2):
                            off = c * VXW + hh * 64
                            for kc in range(2):
                                nc.tensor.matmul(
                                    otp[:, hh * S:(hh + 1) * S],
                                    seg_vx[b2 * 2 + kc][:, off:off + 128],
                                    es2[hh][:, kc * S:(kc + 1) * S],
                                    start=(kc == 0), stop=(kc == 1),
                                    skip_group_check=True)
                        # denominators stay in their own partitions (even
                        # head's at p64, odd head's at p0 -- engines cannot
                        # shift partitions); the selector matmul broadcasts
                        # both rows across partitions, then one reciprocal on
                        # the partition-0-based psum tile (custom DVE ops only
                        # work from partition 0).
                        denf = denf_t[(b2 * NC_D + c) % 2]
                        nc.vector.tensor_copy(denf[64:65, :], otp[64:65, 0:S])
                        nc.vector.tensor_copy(denf[0:1, :], otp[0:1, S:2 * S])
                        db2 = P["pstat"].tile([128, S], F32, tag="rb2", bufs=1)
                        nc.tensor.matmul(db2, selm, denf, start=True, stop=True)
                        rbsf = P["rbs"].tile([128, S], F32, tag="rbs")
                        nc.vector.reciprocal_approx_fast(out=rbsf, in_=db2)
                        nc.vector.tensor_mul(
                            seg_o[c][0:HD, bcol:bcol + S], otp[0:HD, 0:S], rbsf[0:HD, :])
                        nc.vector.tensor_mul(
                            seg_o[c][HD:128, bcol:bcol + S], otp[HD:128, S:2 * S],
                            rbsf[HD:128, :])

            def wo_proj(seg, l_, b):
                xs = [xT[c][seg][:, :] for c in range(NC_D)]
                proj_fm(wo_t[b], [t[:, :] for t in seg_o], xs,
                        (lambda mc: ap_vec("BO", l_ * 4 + mc)) if flags["bo"] else None,
                        resid=True)

            def ffn(seg, l_, b):
                xs = [xT[c][seg][:, :] for c in range(NC_D)]
                for mc in range(NC_FF):
                    ps = P["pbig"].tile([128, TT], F32, tag="pb")
                    for kc in range(NC_D):
                        nc.tensor.matmul(
                            ps, w1_t[b][kc][:, mc * 128:(mc + 1) * 128], xs[kc],
                            start=(kc == 0), stop=(kc == NC_D - 1))
                    nc.scalar.activation(
                        hT[mc][:, :], ps, AF.Gelu,
                        bias=ap_vec("B1", l_ * 16 + mc) if flags["b1"] else 0.0,
                        scale=1.0)
                for mcD in range(NC_D):
                    ps = P["pbig"].tile([128, TT], F32, tag="pb")
                    for kc in range(NC_FF):
                        nc.tensor.matmul(
                            ps, w2_t[b][kc][:, mcD * 128:(mcD + 1) * 128],
                            hT[kc][:, :],
                            start=(kc == 0), stop=(kc == NC_FF - 1))
                    nc.vector.scalar_tensor_tensor(
                        xs[mcD], ps,
                        ap_vec("B2", l_ * 4 + mcD) if flags["b2"] else 0.0,
                        xs[mcD], ALU.add, ALU.add)

            def ln1(seg, l_):
                xs = [xT[c][seg][:, :] for c in range(NC_D)]
                layernorm(xs, D,
                          (lambda c: ap_vec("G1", l_ * 4 + c)) if flags["g1"] else (lambda c: None),
                          (lambda c: ap_vec("BE1", l_ * 4 + c)) if flags["be1"] else (lambda c: None))

            def ln2(seg, l_):
                xs = [xT[c][seg][:, :] for c in range(NC_D)]
                layernorm(xs, D,
                          (lambda c: ap_vec("G2", l_ * 4 + c)) if flags["g2"] else (lambda c: None),
                          (lambda c: ap_vec("BE2", l_ * 4 + c)) if flags["be2"] else (lambda c: None))

            # ---------------- input stage ----------------
            obs_flat = obs_d.rearrange("b s f -> (b s) f")
            obs_t = []
            for i in range(16):
                ot = P["scratch"].tile([128, OBS], BF16, tag="obs_in", bufs=16,
                                       name=f"ot{i}")
                nc.sync.dma_start(out=ot, in_=obs_flat[i * 128:(i + 1) * 128, :])
                obs_t.append(ot)
            nc.sync.dma_start(out=win_t[0:OBS, :], in_=win_d[:, :])
            load_weights(0, 0)
            for seg in range(NSEG):
                obsT = seg_k[0]  # [96, 512] region used
                for ts in range(4):
                    tp = P["ptp"].tile([128, 128], BF16, tag="tpb")
                    nc.tensor.transpose(tp[0:OBS, :], obs_t[seg * 4 + ts], ident)
                    nc.scalar.copy(obsT[0:OBS, ts * 128:(ts + 1) * 128], tp[0:OBS, :])
                xs = [xT[c][seg][:, :] for c in range(NC_D)]
                proj_fm([win_t], [obsT[:, :]], xs,
                        (lambda mc: ap_vec("BIN", mc)) if flags["bin_"] else None,
                        kpart=OBS)
                layernorm(xs, D,
                          (lambda c: ap_vec("GIN", c)) if flags["gin"] else (lambda c: None),
                          (lambda c: ap_vec("BEIN", c)) if flags["bein"] else (lambda c: None),
                          gelu=True)
                for c in range(NC_D):
                    xc = xs[c]
                    nc.vector.tensor_add(
                        xc.rearrange("p (b s) -> p b s", s=S),
                        xc.rearrange("p (b s) -> p b s", s=S),
                        peT[:, c * S:(c + 1) * S].unsqueeze(1).broadcast_to([128, TT // S, S]))

            # ---------------- layers ----------------
            # steady-state emission per seg s:
            #   [attn(s+1)] [LN1(s)] [FFN(s)] [Wo(s+1)] [LN2(s)]
            # with attn/Wo rolling into the next layer at s==3.
            if dbg_att and n_layers > 0:
                attn_block(0, 0, 0)
                for c in range(NC_D):
                    nc.sync.dma_start(out=adbg_d[0, c * 128:(c + 1) * 128, :],
                                      in_=seg_q[c][:, :])
                    nc.sync.dma_start(out=adbg_d[1, c * 128:(c + 1) * 128, :],
                                      in_=seg_k[c][:, :])
                    nc.sync.dma_start(out=adbg_d[2, c * 128:(c + 1) * 128, :],
                                      in_=seg_o[c][:, :])
                for t_ in range(4):
                    nc.sync.dma_start(out=vdbg_d[t_, :, :], in_=seg_vx[t_][:, :])
                for i in range(2):
                    nc.sync.dma_start(out=rdbg_d[i, :, :], in_=denf_t[i][:, :])
            elif n_layers > 0:
                attn_block(0, 0, 0)
                wo_proj(0, 0, 0)
            for l in range(n_layers if not dbg_att else 0):
                b = l % 2
                if l + 1 < n_layers:
                    load_weights(l + 1, 1 - b)
                for s in range(NSEG):
                    ln1(s, l)
                    if s + 1 < NSEG:
                        attn_block(s + 1, l, b)
                    elif l + 1 < n_layers:
                        attn_block(0, l + 1, 1 - b)
                    ffn(s, l, b)
                    if s + 1 < NSEG:
                        wo_proj(s + 1, l, b)
                    elif l + 1 < n_layers:
                        wo_proj(0, l + 1, 1 - b)
                    ln2(s, l)

            if dbg_x:
                for c in range(NC_D):
                    for s in range(NSEG):
                        nc.sync.dma_start(
                            out=xdbg_d[c * 128:(c + 1) * 128, s * TT:(s + 1) * TT],
                            in_=xT[c][s][:, :])

            # ---------------- head (phase-major, 4-seg pipeline) ----------------
            if emit_head:
                for k in range(NC_D):
                    nc.sync.dma_start(out=wp1_t[k], in_=wp1_d[k * 128:(k + 1) * 128, :])
                for k in range(2):
                    nc.sync.dma_start(out=wp2_t[k], in_=wp2_d[k * 128:(k + 1) * 128, :])
                nc.sync.dma_start(out=wp3_t[0:128, :], in_=wp3_d[:, :])
                y1s = [[seg_q[s][:, :], seg_k[s][:, :]] for s in range(NSEG)]
                y2s = [seg_vx[s][:, 0:TT] for s in range(NSEG)]
                for s in range(NSEG):
                    xs = [xT[c][s][:, :] for c in range(NC_D)]
                    proj_fm(wp1_t, xs, y1s[s],
                            (lambda mc: ap_vec("BP1", mc)) if flags["bp1"] else None)
                    layernorm(y1s[s], D // 2,
                              (lambda c: ap_vec("GP1", c)) if flags["gp1"] else (lambda c: None),
                              (lambda c: ap_vec("BEP1", c)) if flags["bep1"] else (lambda c: None),
                              gelu=True)
                for s in range(NSEG):
                    proj_fm(wp2_t, y1s[s], [y2s[s]],
                            (lambda mc: ap_vec("BP2", 0)) if flags["bp2"] else None)
                    layernorm([y2s[s]], D // 4,
                              (lambda c: ap_vec("GP2", 0)) if flags["gp2"] else (lambda c: None),
                              (lambda c: ap_vec("BEP2", 0)) if flags["bep2"] else (lambda c: None),
                              gelu=True)
                for s in range(NSEG):
                    actp = P["pbig"].tile([ACT_DIM, TT], F32, tag="pb")
                    nc.tensor.matmul(actp, wp3_t[0:128, :], y2s[s], start=True, stop=True)
                    actT = P["headt"].tile([ACT_DIM, TT], BF16, tag="actT")
                    nc.scalar.activation(actT[0:ACT_DIM, :], actp, AF.Tanh,
                                         bias=vt["BP3"][0:ACT_DIM, 0:1] if flags["bp3"] else 0.0,
                                         scale=1.0)
                    if flags["asc"] or flags["abi"]:
                        nc.scalar.activation(
                            actT[0:ACT_DIM, :], actT[0:ACT_DIM, :], AF.Identity,
                            bias=vt["ABI"][0:ACT_DIM, 0:1] if flags["abi"] else 0.0,
                            scale=vt["ASC"][0:ACT_DIM, 0:1] if flags["asc"] else 1.0)
                    for ts in range(4):
                        tp = P["ptp"].tile([128, 128], BF16, tag="tpb")
                        nc.tensor.transpose(tp[0:128, 0:ACT_DIM],
                                            actT[0:ACT_DIM, ts * 128:(ts + 1) * 128],
                                            ident[0:ACT_DIM, 0:ACT_DIM])
                        ob = P["scratch"].tile([128, ACT_DIM], F32, tag="ob")
                        nc.vector.tensor_copy(ob, tp[0:128, 0:ACT_DIM])
                        nc.sync.dma_start(
                            out=out_d[s * TT + ts * 128: s * TT + (ts + 1) * 128, :],
                            in_=ob)

    nc.compile()
    return nc, extra


# ======================================================================
# Self-contained kernel entry point: takes FULL inputs, shards batch over
# 8 NeuronCores (data-parallel), runs the Bass kernel, gathers output.
# ======================================================================
from concourse.bass_utils import run_bass_kernel_spmd

N_CORES = 8


def make_in_maps(inputs, extra):
    base = dict(extra)
    for k in ["W_in", "Wq", "Wk", "Wv", "Wo", "W1", "W2", "Wp1", "Wp2", "Wp3"]:
        base[k] = np.ascontiguousarray(np.asarray(inputs[k], np.float32).astype(BF))
    obs = np.asarray(inputs["observations"], np.float32).astype(BF)
    n_b = obs.shape[0]
    per = n_b // N_CORES
    in_maps = []
    for c in range(N_CORES):
        m = dict(base)
        m["observations"] = np.ascontiguousarray(obs[c * per:(c + 1) * per])
        in_maps.append(m)
    return in_maps, per


def kernel(**inputs):
    inputs = {k: np.asarray(v) for k, v in inputs.items()}
    nc, extra = build(inputs, n_layers=8, emit_head=True, dbg_x=False)
    in_maps, per = make_in_maps(inputs, extra)

    last_err = None
    for attempt in range(4):
        try:
            res = run_bass_kernel_spmd(nc, in_maps, core_ids=list(range(N_CORES)),
                                       trace=False)
            outs = [res.results[c]["OUT"].reshape(per, S, ACT_DIM)
                    for c in range(N_CORES)]
            return np.concatenate(outs, axis=0)
        except Exception as e:  # transient NRT_EXEC_UNIT_UNRECOVERABLE etc.
            last_err = e
            import time as _time
            _time.sleep(3.0 * (attempt + 1))
    raise last_err
